# revision 11
# baseline (speedup 1.0000x reference)
"""GAT (graph attention) Bass kernel for TRN2, 8-core SPMD — v3.

Math (equivalent to reference up to fp reassociation):
  feat = x @ W (per head);  el = x @ wl, er = x @ wr (per head)
  g    = feat @ fc_w (per head)
  w_e  = exp(leakyrelu(el[src] + er[dst]))   (softmax without max-subtraction)
  s[d] = sum_{e->d} w_e ;  usum[d] = sum_{e->d} w_e * g[src]
  y[d] = usum[d]/s[d] + bias@fc_w + fc_b

v3 changes vs v2 (963994ns):
  - per-core rotated node table (own dst range first): the own-range er
    values are extracted from the main table pass (no separate er pass /
    xTown input); the rotation is folded into the host-side src remap.
  - dofr (replicated dst-offset rows, 46MB HBM/core) eliminated: only the
    S one-hot is built by is_eq; ST comes from a PE transpose of S
    (identity-rhs matmul) with the PSUM->SBUF cast on the idle Scalar
    engine.
  - DMA batching: 32-tile x loads (1MB each), a single up-front A_idx
    load, one row_table store per batch, one y store per window group
    (y padded to NG*GROUPW*WIN rows, host trims) — cuts the sync
    sequencer's serial DMA dispatch cost ~9x.
  - phase-1 PSUM->SBUF row casts moved to the Scalar engine (batched 4
    tiles per op); rsb pad memset moved to GpSimd (idle in phase 1).

Sharding: dst-range partitioning. Core k owns dst nodes
[k*NPC, (k+1)*NPC). Each core computes the bf16 node row table [el|g] for
all N nodes (rows padded to 256B) in rotated order (own range first),
gathers src rows per edge with dma_gather, and scatter-adds via one-hot
matmuls into PSUM window accumulators (128 dst nodes per window, GROUPW
windows per PSUM bank).

Edges (host-side index prep only) are bucketed by (core, window, src-half);
the src-half split keeps gather indices < 32768 (int16 limit).
"""

import numpy as np

import concourse.bass as bass
import concourse.mybir as mybir
import concourse.tile as tile
from concourse import bacc, library_config

F32 = mybir.dt.float32
BF16 = mybir.dt.bfloat16
I16 = mybir.dt.int16
ALU = mybir.AluOpType
ACTF = mybir.ActivationFunctionType

NEG_SLOPE = 0.2
NQUEUES = 4  # SWDGE queues for gather desc-gen


class Cfg:
    def __init__(self, N=50000, E=1200000, CT=10, GROUPW=10):
        self.N = N
        self.E = E
        self.IN = 256           # input feature dim
        self.H = 4              # heads
        self.O = 64             # per-head out dim
        self.D = 10             # final fc dim
        self.NCORES = 8
        self.WIN = 128          # dst nodes per window
        self.GROUPW = GROUPW    # windows per PSUM accumulator tile
        self.CT = CT            # tiles per gather call
        assert CT * 8 + 1 <= 256  # SWDGE ring: descs per lane per call
        assert N % self.NCORES == 0
        self.NPC = N // self.NCORES
        self.NG = -(-(-(-self.NPC // self.WIN)) // GROUPW)
        self.NW = self.NG * GROUPW  # padded to full groups (empty tail ok)
        assert N % 2 == 0
        self.SPLIT = N // 2
        assert self.SPLIT <= 32767 and self.NPC <= 32767
        self.KI = -(-self.IN // 128)           # input-dim chunks (2)
        self.KHO = -(-(self.H * self.O) // 128)  # head*out chunks (2)
        self.AUXW = 2 * self.H + self.H * self.D  # 48: [er|el|g]
        self.ROWW = 128                         # bf16 row width = 256B
        self.HD = self.H * self.D               # 40
        self.MAINW = self.H + self.HD           # 44: [el|g] payload width
        self.NB = 16            # node tiles per phase-1 load batch
        self.SB = 4             # tiles per phase-1 PSUM sub-batch
        self.NYR = self.NG * GROUPW * self.WIN  # padded y rows per core


class Structure:
    """Compile-time stream structure shared by host packer and program builder."""

    def __init__(self, cfg: Cfg, T_wh: np.ndarray):
        self.T_wh = T_wh  # [NW, 2] tiles per (window, half)
        tile_meta = []    # (w, half, group_first, group_last)
        call_meta = []    # (tile_start, ntiles, half)
        pos_wh = np.zeros((cfg.NW, 2), np.int64)
        for g in range(cfg.NG):
            ws = range(g * cfg.GROUPW, min((g + 1) * cfg.GROUPW, cfg.NW))
            g_first_tile = len(tile_meta)
            g_ntiles = int(T_wh[list(ws), :].sum())
            for half in (0, 1):
                run_start = len(tile_meta)
                for w in ws:
                    pos_wh[w, half] = len(tile_meta)
                    for t in range(T_wh[w, half]):
                        first = (len(tile_meta) == g_first_tile)
                        last = (len(tile_meta) == g_first_tile + g_ntiles - 1)
                        tile_meta.append((w, half, first, last))
                run_len = len(tile_meta) - run_start
                o = 0
                while o < run_len:
                    c = min(cfg.CT, run_len - o)
                    call_meta.append((run_start + o, c, half))
                    o += c
        self.tile_meta = tile_meta
        self.call_meta = call_meta
        self.pos_wh = pos_wh
        self.NT = len(tile_meta)


def preprocess(cfg: Cfg, src: np.ndarray, dst: np.ndarray):
    """Host-side index/layout prep (no float arithmetic).

    Returns (structure, A_idx[NC,NPOS] i16, doff[NC,NPOS] f32)."""
    src = np.asarray(src, np.int64)
    dst = np.asarray(dst, np.int64)
    core = dst // cfg.NPC
    dstl = dst - core * cfg.NPC
    w = dstl // cfg.WIN
    off = dstl - w * cfg.WIN
    # per-core rotation: core c's table row r holds node (c*NPC + r) mod N
    srcr_all = (src - core * cfg.NPC) % cfg.N
    half = (srcr_all >= cfg.SPLIT).astype(np.int64)
    srcr = srcr_all - half * cfg.SPLIT

    key = (core * cfg.NW + w) * 2 + half
    order = np.argsort(key, kind="stable")
    nk = cfg.NCORES * cfg.NW * 2
    cnt = np.bincount(key, minlength=nk)
    starts = np.zeros(nk + 1, np.int64)
    np.cumsum(cnt, out=starts[1:])
    cnt_cwh = cnt.reshape(cfg.NCORES, cfg.NW, 2)
    T_wh = np.maximum(-(-cnt_cwh.max(axis=0) // cfg.WIN), 1)  # [NW,2]
    st = Structure(cfg, T_wh)

    NPOS = st.NT * 128
    A_idx = np.zeros((cfg.NCORES, NPOS), np.int16)
    doff = np.full((cfg.NCORES, NPOS), -1.0, np.float32)
    for c in range(cfg.NCORES):
        for wv in range(cfg.NW):
            for h in (0, 1):
                kk = (c * cfg.NW + wv) * 2 + h
                n = cnt[kk]
                if n == 0:
                    continue
                eids = order[starts[kk]:starts[kk] + n]
                p0 = st.pos_wh[wv, h] * 128
                A_idx[c, p0:p0 + n] = srcr[eids]
                doff[c, p0:p0 + n] = off[eids]
    return st, A_idx, doff


def host_layouts(cfg: Cfg, x, W, attn_l, attn_r, bias, fc_w, fc_b):
    """Pure layout transforms of the inputs (no model arithmetic)."""
    import ml_dtypes
    bf = ml_dtypes.bfloat16
    H, O, D, IN = cfg.H, cfg.O, cfg.D, cfg.IN
    xT = np.ascontiguousarray(np.asarray(x, np.float32).T).astype(bf)  # [IN, N]
    WT = np.ascontiguousarray(
        np.asarray(W, np.float32).transpose(0, 2, 1).reshape(H * O, IN))
    wcat = np.zeros((H * O, cfg.AUXW), np.float32)  # [ho, er|el|g]
    for h in range(H):
        r = slice(h * O, (h + 1) * O)
        wcat[r, h] = attn_r[h]
        wcat[r, H + h] = attn_l[h]
        wcat[r, 2 * H + h * D:2 * H + (h + 1) * D] = fc_w
    bias_flat = np.asarray(bias, np.float32).reshape(H * O, 1)
    fcb_tiled = np.tile(np.asarray(fc_b, np.float32), H).reshape(1, H * D)
    return xT, WT, wcat, bias_flat, fcb_tiled


def build_program(cfg: Cfg, st: Structure):
    nc = bacc.Bacc(trn_type="TRN2", num_swdge_queues=NQUEUES,
                   dynamic_dma_scratch_size=16384)
    N, IN, H, O, D = cfg.N, cfg.IN, cfg.H, cfg.O, cfg.D
    KI, KHO, AUXW, ROWW, HD, MAINW = (cfg.KI, cfg.KHO, cfg.AUXW, cfg.ROWW,
                                      cfg.HD, cfg.MAINW)
    WIN, NW, NG, GROUPW, NPC, CT = (cfg.WIN, cfg.NW, cfg.NG, cfg.GROUPW,
                                    cfg.NPC, cfg.CT)
    NB, SB = cfg.NB, cfg.SB
    NT = st.NT

    xT = nc.dram_tensor("xT", [IN, N], BF16, kind="ExternalInput")
    WTt = nc.dram_tensor("WT", [H * O, IN], F32, kind="ExternalInput")
    wcat_t = nc.dram_tensor("wcat", [H * O, AUXW], F32, kind="ExternalInput")
    bias_t = nc.dram_tensor("bias_flat", [H * O, 1], F32, kind="ExternalInput")
    fcb_t = nc.dram_tensor("fcb_tiled", [1, HD], F32, kind="ExternalInput")
    Aidx_t = nc.dram_tensor("A_idx", [128, NT * 8], I16, kind="ExternalInput")
    dofc_t = nc.dram_tensor("dofc", [128, NT], BF16, kind="ExternalInput")
    iota_t = nc.dram_tensor("iota_rep", [128, CT * WIN], BF16,
                            kind="ExternalInput")
    ident_t = nc.dram_tensor("ident", [128, 128], BF16, kind="ExternalInput")
    y_t = nc.dram_tensor("y", [cfg.NYR, HD], F32, kind="ExternalOutput")

    row_table = nc.dram_tensor("row_table", [N, ROWW], BF16, kind="Internal")

    with tile.TileContext(nc) as tc, \
            tc.tile_pool(name="const", bufs=1) as cp, \
            tc.tile_pool(name="p1", bufs=2) as p1, \
            tc.tile_pool(name="p1ps", bufs=2, space="PSUM") as p1ps, \
            tc.tile_pool(name="stps", bufs=2, space="PSUM") as stpsp, \
            tc.tile_pool(name="gath", bufs=8) as gp, \
            tc.tile_pool(name="tp", bufs=2) as tp, \
            tc.tile_pool(name="erps", bufs=2, space="PSUM") as erps, \
            tc.tile_pool(name="acc", bufs=2, space="PSUM") as accp, \
            tc.tile_pool(name="outp", bufs=2) as op:

        # ---------- phase 0: constants ----------
        wt_sb = cp.tile([128, KHO, IN], F32)
        wcat_sb = cp.tile([128, KHO, AUXW], F32)
        bf_sb = cp.tile([128, KHO, 1], F32)
        for a in range(KHO):
            r = slice(a * 128, (a + 1) * 128)
            nc.sync.dma_start(out=wt_sb[:, a, :], in_=WTt[r, :])
            nc.sync.dma_start(out=wcat_sb[:, a, :], in_=wcat_t[r, :])
            nc.sync.dma_start(out=bf_sb[:, a, :], in_=bias_t[r, :])
        fcb_sb = cp.tile([1, HD], F32)
        nc.sync.dma_start(out=fcb_sb[:], in_=fcb_t[:])

        # aux projection matrix [IN, er|el|g] = WT.T @ wcat, then bf16
        aux_bf = cp.tile([128, KI, AUXW], BF16)
        for m in range(KI):
            aps = p1ps.tile([128, AUXW], F32, tag="rps")
            for k in range(KHO):
                nc.tensor.matmul(out=aps[:], lhsT=wt_sb[:, k, m * 128:(m + 1) * 128],
                                 rhs=wcat_sb[:, k, :], start=(k == 0), stop=(k == KHO - 1))
            nc.vector.tensor_copy(out=aux_bf[:, m, :], in_=aps[:])

        # bias@fc_w + fc_b, replicated to 128 partitions and GROUPW windows
        brow_ps = p1ps.tile([1, HD], F32, tag="rps")
        for k in range(KHO):
            nc.tensor.matmul(out=brow_ps[:], lhsT=bf_sb[:, k, :],
                             rhs=wcat_sb[:, k, 2 * H:AUXW],
                             start=(k == 0), stop=(k == KHO - 1))
        brow_sb = cp.tile([1, HD], F32)
        nc.vector.tensor_add(out=brow_sb[:], in0=brow_ps[:], in1=fcb_sb[:])
        ones_sb = cp.tile([1, 128], F32)
        nc.vector.memset(ones_sb[:], 1.0)
        brep_ps = p1ps.tile([128, HD], F32, tag="rps")
        nc.tensor.matmul(out=brep_ps[:], lhsT=ones_sb[:], rhs=brow_sb[:],
                         start=True, stop=True)
        brep_sb = cp.tile([128, GROUPW * HD], F32)
        for wl in range(GROUPW):
            nc.vector.tensor_copy(out=brep_sb[:, wl * HD:(wl + 1) * HD], in_=brep_ps[:])

        iota_f = cp.tile([128, CT * WIN], BF16)
        nc.sync.dma_start(out=iota_f[:], in_=iota_t[:])
        ident_sb = cp.tile([128, 128], BF16)
        nc.sync.dma_start(out=ident_sb[:], in_=ident_t[:])
        dofc_sb = cp.tile([128, NT], BF16)
        nc.sync.dma_start(out=dofc_sb[:], in_=dofc_t[:])
        aidx_sb = cp.tile([128, NT * 8], I16)
        nc.sync.dma_start(out=aidx_sb[:], in_=Aidx_t[:])

        # er table for own dst range: window w's 128 nodes down partitions
        er_sb = cp.tile([128, NW * H], BF16)
        nc.vector.memset(er_sb[:], 0.0)

        # ---------- phase 1: node row table (bf16) + own-range er ----------
        ntiles = -(-N // 128)
        for b0 in range(0, ntiles, NB):
            bt = min(NB, ntiles - b0)
            n0 = b0 * 128
            bcnt = min(NB * 128, N - n0)
            xt = p1.tile([128, KI, NB * 128], BF16, tag="xt")
            for k in range(KI):
                nc.sync.dma_start(out=xt[:, k, :bcnt],
                                  in_=xT[k * 128:(k + 1) * 128, n0:n0 + bcnt])
            rsb = p1.tile([128, NB, ROWW], BF16, tag="rsb")
            nc.gpsimd.memset(rsb[:, :, MAINW:], 0)
            for j0 in range(0, bt, SB):
                sb = min(SB, bt - j0)
                rps = p1ps.tile([128, SB, AUXW], F32, tag="rps")
                for jj in range(sb):
                    j = j0 + jj
                    cnt = min(128, N - (b0 + j) * 128)
                    for k in range(KI):
                        nc.tensor.matmul(
                            out=rps[:cnt, jj, :],
                            lhsT=xt[:, k, j * 128:j * 128 + cnt],
                            rhs=aux_bf[:, k, :],
                            start=(k == 0), stop=(k == KI - 1))
                # main row payload: [el|g] = aux cols H..AUXW (Scalar engine)
                nc.scalar.activation(out=rsb[:, j0:j0 + sb, :MAINW],
                                     in_=rps[:, :sb, H:AUXW], func=ACTF.Copy)
                # own-range er extraction (rotated: own dst range = tiles 0..NW-1)
                t0 = b0 + j0
                if t0 < NW:
                    nt_er = min(sb, NW - t0)
                    nc.scalar.activation(
                        out=er_sb[:, t0 * H:(t0 + nt_er) * H]
                            .rearrange("p (w h) -> p w h", h=H),
                        in_=rps[:, :nt_er, 0:H], func=ACTF.Copy)
            if bcnt == bt * 128:
                out_ap = row_table[n0:n0 + bt * 128, :].rearrange(
                    "(j p) c -> p j c", p=128)
                nc.sync.dma_start(out=out_ap, in_=rsb[:, :bt, :])
            else:
                full = bcnt // 128
                if full:
                    out_ap = row_table[n0:n0 + full * 128, :].rearrange(
                        "(j p) c -> p j c", p=128)
                    nc.sync.dma_start(out=out_ap, in_=rsb[:, :full, :])
                for j in range(full, bt):
                    cnt = min(128, N - (b0 + j) * 128)
                    nc.sync.dma_start(
                        out=row_table[(b0 + j) * 128:(b0 + j) * 128 + cnt, :],
                        in_=rsb[:cnt, j, :])

        # ---------- phase 2: edge stream ----------
        cur_g = [-1]
        gps_ref = [None]

        def close_group(g):
            gps = gps_ref[0]
            gv = gps[:].rearrange("p (w c) -> p w c", c=MAINW)
            sg = op.tile([128, GROUPW * H], F32, tag="sg")
            nc.vector.tensor_scalar_max(out=sg[:], in0=gv[:, :, 0:H],
                                        scalar1=1e-30)
            rs = op.tile([128, GROUPW * H], F32, tag="rs")
            nc.vector.reciprocal(out=rs[:], in_=sg[:])
            ysb = op.tile([128, GROUPW * HD], F32, tag="ysb")
            nc.vector.tensor_tensor(
                out=ysb[:].rearrange("p (w h d) -> p w h d", h=H, d=D),
                in0=gv[:, :, H:MAINW].rearrange("p w (h d) -> p w h d", h=H),
                in1=rs[:].rearrange("p (w h) -> p w h", h=H)
                    .to_broadcast([128, GROUPW, H, D]),
                op=ALU.mult)
            nc.vector.tensor_add(out=ysb[:], in0=ysb[:], in1=brep_sb[:])
            n0 = g * GROUPW * WIN
            out_ap = y_t[n0:n0 + GROUPW * WIN, :].rearrange(
                "(w p) c -> p w c", p=128)
            nc.sync.dma_start(out=out_ap, in_=ysb[:].rearrange(
                "p (w c) -> p w c", c=HD))

        nreg_cache = {}

        def nreg(n):
            if n not in nreg_cache:
                nreg_cache[n] = nc.gpsimd.to_reg(n)
            return nreg_cache[n]

        gq = [0]  # rotating SWDGE queue counter

        for (c0, ctiles, half) in st.call_meta:
            ne = ctiles * 128
            abuf = gp.tile([128, CT, ROWW], BF16, tag="abuf")
            tab = row_table[half * cfg.SPLIT:(half + 1) * cfg.SPLIT, :]
            nc.gpsimd.dma_gather(abuf[:, :ctiles, :], tab,
                                 aidx_sb[:, c0 * 8:(c0 + ctiles) * 8],
                                 ctiles * 128, nreg(ctiles * 128), ROWW,
                                 queue_num=gq[0] % NQUEUES,
                                 single_packet=False)
            gq[0] += 1

            # batched one-hot S for this call (bf16)
            S_all = tp.tile([128, CT, WIN], BF16, tag="S")
            nc.vector.tensor_tensor(
                out=S_all[:, :ctiles, :],
                in0=dofc_sb[:, c0:c0 + ctiles]
                    .rearrange("p (t o) -> p t o", o=1)
                    .to_broadcast([128, ctiles, WIN]),
                in1=iota_f[:, :ctiles * WIN].rearrange("p (t w) -> p t w", w=WIN),
                op=ALU.is_equal)

            # quad-batched: ST = S^T via PE into one PSUM bank, one Scalar
            # cast per quad, then per-tile er fetch matmuls
            er_ps = erps.tile([128, CT * H], F32, tag="erps")
            for q0 in range(0, ctiles, 4):
                qn = min(4, ctiles - q0)
                st_ps = stpsp.tile([128, 4, 128], F32, tag="stps")
                for jj in range(qn):
                    nc.tensor.matmul(out=st_ps[:, jj, :],
                                     lhsT=S_all[:, q0 + jj, :],
                                     rhs=ident_sb[:], start=True, stop=True)
                st_sb = tp.tile([128, 4, 128], BF16, tag="stsb")
                nc.scalar.activation(out=st_sb[:, :qn, :], in_=st_ps[:, :qn, :],
                                     func=ACTF.Copy)
                for jj in range(qn):
                    j = q0 + jj
                    wv = st.tile_meta[c0 + j][0]
                    nc.tensor.matmul(out=er_ps[:, j * H:(j + 1) * H],
                                     lhsT=st_sb[:, jj, :],
                                     rhs=er_sb[:, wv * H:(wv + 1) * H],
                                     start=True, stop=True)

            # logits -> lrelu -> exp -> mgc (batched)
            esb = tp.tile([128, CT * H], BF16, tag="esb")
            nc.vector.tensor_tensor(
                out=esb[:, :ctiles * H].rearrange("p (t h) -> p t h", h=H),
                in0=abuf[:, :ctiles, 0:H],
                in1=er_ps[:, :ctiles * H].rearrange("p (t h) -> p t h", h=H),
                op=ALU.add)
            nc.vector.scalar_tensor_tensor(
                out=esb[:, :ctiles * H], in0=esb[:, :ctiles * H],
                scalar=NEG_SLOPE, in1=esb[:, :ctiles * H],
                op0=ALU.mult, op1=ALU.max)
            mgc = tp.tile([128, CT, MAINW], BF16, tag="mgc")
            nc.scalar.activation(out=mgc[:, :ctiles, 0:H],
                                 in_=esb[:, :ctiles * H]
                                 .rearrange("p (t h) -> p t h", h=H),
                                 func=ACTF.Exp)
            nc.vector.tensor_tensor(
                out=mgc[:, :ctiles, H:MAINW].rearrange("p t (h d) -> p t h d", h=H),
                in0=abuf[:, :ctiles, H:MAINW].rearrange("p t (h d) -> p t h d", h=H),
                in1=mgc[:, :ctiles, 0:H].to_broadcast([128, ctiles, H, D]),
                op=ALU.mult)

            # per-tile scatter matmuls into window-group accumulators
            for j in range(ctiles):
                tg = c0 + j
                wv, half_, first, last = st.tile_meta[tg]
                g = wv // GROUPW
                if g != cur_g[0]:
                    if cur_g[0] >= 0:
                        close_group(cur_g[0])
                    gps_ref[0] = accp.tile([128, GROUPW * MAINW], F32, tag="gps",
                                           name="gps")
                    cur_g[0] = g
                gps = gps_ref[0]
                wloc = wv - g * GROUPW
                base = wloc * MAINW
                nc.tensor.matmul(out=gps[:, base:base + MAINW],
                                 lhsT=S_all[:, j, :], rhs=mgc[:, j, :],
                                 start=first, stop=last)
        close_group(cur_g[0])

    nc.compile()
    return nc


def make_in_maps(cfg, st, inputs, A_idx, doff):
    import ml_dtypes
    bf = ml_dtypes.bfloat16
    x = np.asarray(inputs["x"], np.float32)
    xT, WT, wcat, bias_flat, fcb_tiled = host_layouts(
        cfg, x, inputs["W"], inputs["attn_l"], inputs["attn_r"],
        inputs["bias"], inputs["fc_w"], inputs["fc_b"])

    def wrap16(a):  # [NPOS] -> [128, NPOS//16]
        return np.tile(np.ascontiguousarray(a.reshape(-1, 16).T), (8, 1))

    in_maps = []
    for c in range(cfg.NCORES):
        dof = doff[c]
        in_maps.append({
            "xT": np.ascontiguousarray(np.roll(xT, -c * cfg.NPC, axis=1)),
            "WT": WT, "wcat": wcat, "bias_flat": bias_flat,
            "fcb_tiled": fcb_tiled,
            "A_idx": wrap16(A_idx[c]),
            "dofc": np.ascontiguousarray(dof.reshape(-1, 128).T).astype(bf),
            "iota_rep": np.tile(np.arange(cfg.WIN), (128, cfg.CT)).astype(bf),
            "ident": np.eye(128, dtype=np.float32).astype(bf),
        })
    return in_maps


def kernel(**inputs):
    import numpy as np
    from concourse import bass_utils

    cfg = Cfg()
    src = np.asarray(inputs["src"])
    dst = np.asarray(inputs["dst"])
    assert src.shape == (cfg.E,) and dst.shape == (cfg.E,)
    st, A_idx, doff = preprocess(cfg, src, dst)
    nc = build_program(cfg, st)
    in_maps = make_in_maps(cfg, st, inputs, A_idx, doff)
    res = bass_utils.run_bass_kernel_spmd(
        nc, in_maps, core_ids=list(range(cfg.NCORES)))
    y = np.concatenate([r["y"][:cfg.NPC] for r in res.results], axis=0)
    return np.ascontiguousarray(y.reshape(cfg.N, cfg.H, cfg.D).astype(np.float32))


# revision 12
# speedup vs baseline: 1.0527x; 1.0527x over previous
"""GAT (graph attention) Bass kernel for TRN2, 8-core SPMD — v3.

Math (equivalent to reference up to fp reassociation):
  feat = x @ W (per head);  el = x @ wl, er = x @ wr (per head)
  g    = feat @ fc_w (per head)
  w_e  = exp(leakyrelu(el[src] + er[dst]))   (softmax without max-subtraction)
  s[d] = sum_{e->d} w_e ;  usum[d] = sum_{e->d} w_e * g[src]
  y[d] = usum[d]/s[d] + bias@fc_w + fc_b

v3 changes vs v2 (963994ns):
  - per-core rotated node table (own dst range first): the own-range er
    values are extracted from the main table pass (no separate er pass /
    xTown input); the rotation is folded into the host-side src remap.
  - dofr (replicated dst-offset rows, 46MB HBM/core) eliminated: only the
    S one-hot is built by is_eq; ST comes from a PE transpose of S
    (identity-rhs matmul) with the PSUM->SBUF cast on the idle Scalar
    engine.
  - DMA batching: 32-tile x loads (1MB each), a single up-front A_idx
    load, one row_table store per batch, one y store per window group
    (y padded to NG*GROUPW*WIN rows, host trims) — cuts the sync
    sequencer's serial DMA dispatch cost ~9x.
  - phase-1 PSUM->SBUF row casts moved to the Scalar engine (batched 4
    tiles per op); rsb pad memset moved to GpSimd (idle in phase 1).

Sharding: dst-range partitioning. Core k owns dst nodes
[k*NPC, (k+1)*NPC). Each core computes the bf16 node row table [el|g] for
all N nodes (rows padded to 256B) in rotated order (own range first),
gathers src rows per edge with dma_gather, and scatter-adds via one-hot
matmuls into PSUM window accumulators (128 dst nodes per window, GROUPW
windows per PSUM bank).

Edges (host-side index prep only) are bucketed by (core, window, src-half);
the src-half split keeps gather indices < 32768 (int16 limit).
"""

import numpy as np

import concourse.bass as bass
import concourse.mybir as mybir
import concourse.tile as tile
from concourse import bacc, library_config

F32 = mybir.dt.float32
BF16 = mybir.dt.bfloat16
I16 = mybir.dt.int16
ALU = mybir.AluOpType
ACTF = mybir.ActivationFunctionType

NEG_SLOPE = 0.2
NQUEUES = 4  # SWDGE queues for gather desc-gen


class Cfg:
    def __init__(self, N=50000, E=1200000, CT=20, GROUPW=10):
        self.N = N
        self.E = E
        self.IN = 256           # input feature dim
        self.H = 4              # heads
        self.O = 64             # per-head out dim
        self.D = 10             # final fc dim
        self.NCORES = 8
        self.WIN = 128          # dst nodes per window
        self.GROUPW = GROUPW    # windows per PSUM accumulator tile
        self.CT = CT            # tiles per gather call
        assert CT * 8 + 1 <= 256  # SWDGE ring: descs per lane per call
        assert N % self.NCORES == 0
        self.NPC = N // self.NCORES
        self.NG = -(-(-(-self.NPC // self.WIN)) // GROUPW)
        self.NW = self.NG * GROUPW  # padded to full groups (empty tail ok)
        assert N % 2 == 0
        self.SPLIT = N // 2
        assert self.SPLIT <= 32767 and self.NPC <= 32767
        self.KI = -(-self.IN // 128)           # input-dim chunks (2)
        self.KHO = -(-(self.H * self.O) // 128)  # head*out chunks (2)
        self.AUXW = 2 * self.H + self.H * self.D  # 48: [er|el|g]
        self.ROWW = 128                         # bf16 row width = 256B
        self.HD = self.H * self.D               # 40
        self.MAINW = self.H + self.HD           # 44: [el|g] payload width
        self.NB = 16            # node tiles per phase-1 load batch
        self.SB = 4             # tiles per phase-1 PSUM sub-batch
        self.NYR = self.NG * GROUPW * self.WIN  # padded y rows per core


class Structure:
    """Compile-time stream structure shared by host packer and program builder."""

    def __init__(self, cfg: Cfg, T_wh: np.ndarray):
        self.T_wh = T_wh  # [NW, 2] tiles per (window, half)
        tile_meta = []    # (w, half, group_first, group_last)
        call_meta = []    # (tile_start, ntiles, half)
        pos_wh = np.zeros((cfg.NW, 2), np.int64)
        for g in range(cfg.NG):
            ws = range(g * cfg.GROUPW, min((g + 1) * cfg.GROUPW, cfg.NW))
            g_first_tile = len(tile_meta)
            g_ntiles = int(T_wh[list(ws), :].sum())
            for half in (0, 1):
                run_start = len(tile_meta)
                for w in ws:
                    pos_wh[w, half] = len(tile_meta)
                    for t in range(T_wh[w, half]):
                        first = (len(tile_meta) == g_first_tile)
                        last = (len(tile_meta) == g_first_tile + g_ntiles - 1)
                        tile_meta.append((w, half, first, last))
                run_len = len(tile_meta) - run_start
                o = 0
                while o < run_len:
                    c = min(cfg.CT, run_len - o)
                    call_meta.append((run_start + o, c, half))
                    o += c
        self.tile_meta = tile_meta
        self.call_meta = call_meta
        self.pos_wh = pos_wh
        self.NT = len(tile_meta)


def preprocess(cfg: Cfg, src: np.ndarray, dst: np.ndarray):
    """Host-side index/layout prep (no float arithmetic).

    Returns (structure, A_idx[NC,NPOS] i16, doff[NC,NPOS] f32)."""
    src = np.asarray(src, np.int64)
    dst = np.asarray(dst, np.int64)
    core = dst // cfg.NPC
    dstl = dst - core * cfg.NPC
    w = dstl // cfg.WIN
    off = dstl - w * cfg.WIN
    # per-core rotation: core c's table row r holds node (c*NPC + r) mod N
    srcr_all = (src - core * cfg.NPC) % cfg.N
    half = (srcr_all >= cfg.SPLIT).astype(np.int64)
    srcr = srcr_all - half * cfg.SPLIT

    key = (core * cfg.NW + w) * 2 + half
    order = np.argsort(key, kind="stable")
    nk = cfg.NCORES * cfg.NW * 2
    cnt = np.bincount(key, minlength=nk)
    starts = np.zeros(nk + 1, np.int64)
    np.cumsum(cnt, out=starts[1:])
    cnt_cwh = cnt.reshape(cfg.NCORES, cfg.NW, 2)
    T_wh = np.maximum(-(-cnt_cwh.max(axis=0) // cfg.WIN), 1)  # [NW,2]
    st = Structure(cfg, T_wh)

    NPOS = st.NT * 128
    A_idx = np.zeros((cfg.NCORES, NPOS), np.int16)
    doff = np.full((cfg.NCORES, NPOS), -1.0, np.float32)
    for c in range(cfg.NCORES):
        for wv in range(cfg.NW):
            for h in (0, 1):
                kk = (c * cfg.NW + wv) * 2 + h
                n = cnt[kk]
                if n == 0:
                    continue
                eids = order[starts[kk]:starts[kk] + n]
                p0 = st.pos_wh[wv, h] * 128
                A_idx[c, p0:p0 + n] = srcr[eids]
                doff[c, p0:p0 + n] = off[eids]
    return st, A_idx, doff


def host_layouts(cfg: Cfg, x, W, attn_l, attn_r, bias, fc_w, fc_b):
    """Pure layout transforms of the inputs (no model arithmetic)."""
    import ml_dtypes
    bf = ml_dtypes.bfloat16
    H, O, D, IN = cfg.H, cfg.O, cfg.D, cfg.IN
    xT = np.ascontiguousarray(np.asarray(x, np.float32).T).astype(bf)  # [IN, N]
    WT = np.ascontiguousarray(
        np.asarray(W, np.float32).transpose(0, 2, 1).reshape(H * O, IN))
    wcat = np.zeros((H * O, cfg.AUXW), np.float32)  # [ho, er|el|g]
    for h in range(H):
        r = slice(h * O, (h + 1) * O)
        wcat[r, h] = attn_r[h]
        wcat[r, H + h] = attn_l[h]
        wcat[r, 2 * H + h * D:2 * H + (h + 1) * D] = fc_w
    bias_flat = np.asarray(bias, np.float32).reshape(H * O, 1)
    fcb_tiled = np.tile(np.asarray(fc_b, np.float32), H).reshape(1, H * D)
    return xT, WT, wcat, bias_flat, fcb_tiled


def build_program(cfg: Cfg, st: Structure):
    nc = bacc.Bacc(trn_type="TRN2", num_swdge_queues=NQUEUES,
                   dynamic_dma_scratch_size=16384)
    N, IN, H, O, D = cfg.N, cfg.IN, cfg.H, cfg.O, cfg.D
    KI, KHO, AUXW, ROWW, HD, MAINW = (cfg.KI, cfg.KHO, cfg.AUXW, cfg.ROWW,
                                      cfg.HD, cfg.MAINW)
    WIN, NW, NG, GROUPW, NPC, CT = (cfg.WIN, cfg.NW, cfg.NG, cfg.GROUPW,
                                    cfg.NPC, cfg.CT)
    NB, SB = cfg.NB, cfg.SB
    NT = st.NT

    xT = nc.dram_tensor("xT", [IN, N], BF16, kind="ExternalInput")
    WTt = nc.dram_tensor("WT", [H * O, IN], F32, kind="ExternalInput")
    wcat_t = nc.dram_tensor("wcat", [H * O, AUXW], F32, kind="ExternalInput")
    bias_t = nc.dram_tensor("bias_flat", [H * O, 1], F32, kind="ExternalInput")
    fcb_t = nc.dram_tensor("fcb_tiled", [1, HD], F32, kind="ExternalInput")
    Aidx_t = nc.dram_tensor("A_idx", [128, NT * 8], I16, kind="ExternalInput")
    dofc_t = nc.dram_tensor("dofc", [128, NT], BF16, kind="ExternalInput")
    iota_t = nc.dram_tensor("iota_rep", [128, CT * WIN], BF16,
                            kind="ExternalInput")
    ident_t = nc.dram_tensor("ident", [128, 128], BF16, kind="ExternalInput")
    y_t = nc.dram_tensor("y", [cfg.NYR, HD], F32, kind="ExternalOutput")

    row_table = nc.dram_tensor("row_table", [N, ROWW], BF16, kind="Internal")

    with tile.TileContext(nc) as tc, \
            tc.tile_pool(name="const", bufs=1) as cp, \
            tc.tile_pool(name="p1", bufs=2) as p1, \
            tc.tile_pool(name="p1ps", bufs=2, space="PSUM") as p1ps, \
            tc.tile_pool(name="stps", bufs=2, space="PSUM") as stpsp, \
            tc.tile_pool(name="gath", bufs=8) as gp, \
            tc.tile_pool(name="tp", bufs=2) as tp, \
            tc.tile_pool(name="erps", bufs=2, space="PSUM") as erps, \
            tc.tile_pool(name="acc", bufs=2, space="PSUM") as accp, \
            tc.tile_pool(name="outp", bufs=2) as op:

        # ---------- phase 0: constants ----------
        wt_sb = cp.tile([128, KHO, IN], F32)
        wcat_sb = cp.tile([128, KHO, AUXW], F32)
        bf_sb = cp.tile([128, KHO, 1], F32)
        for a in range(KHO):
            r = slice(a * 128, (a + 1) * 128)
            nc.sync.dma_start(out=wt_sb[:, a, :], in_=WTt[r, :])
            nc.sync.dma_start(out=wcat_sb[:, a, :], in_=wcat_t[r, :])
            nc.sync.dma_start(out=bf_sb[:, a, :], in_=bias_t[r, :])
        fcb_sb = cp.tile([1, HD], F32)
        nc.sync.dma_start(out=fcb_sb[:], in_=fcb_t[:])

        # aux projection matrix [IN, er|el|g] = WT.T @ wcat, then bf16
        aux_bf = cp.tile([128, KI, AUXW], BF16)
        for m in range(KI):
            aps = p1ps.tile([128, AUXW], F32, tag="rps")
            for k in range(KHO):
                nc.tensor.matmul(out=aps[:], lhsT=wt_sb[:, k, m * 128:(m + 1) * 128],
                                 rhs=wcat_sb[:, k, :], start=(k == 0), stop=(k == KHO - 1))
            nc.vector.tensor_copy(out=aux_bf[:, m, :], in_=aps[:])

        # bias@fc_w + fc_b, replicated to 128 partitions and GROUPW windows
        brow_ps = p1ps.tile([1, HD], F32, tag="rps")
        for k in range(KHO):
            nc.tensor.matmul(out=brow_ps[:], lhsT=bf_sb[:, k, :],
                             rhs=wcat_sb[:, k, 2 * H:AUXW],
                             start=(k == 0), stop=(k == KHO - 1))
        brow_sb = cp.tile([1, HD], F32)
        nc.vector.tensor_add(out=brow_sb[:], in0=brow_ps[:], in1=fcb_sb[:])
        ones_sb = cp.tile([1, 128], F32)
        nc.vector.memset(ones_sb[:], 1.0)
        brep_ps = p1ps.tile([128, HD], F32, tag="rps")
        nc.tensor.matmul(out=brep_ps[:], lhsT=ones_sb[:], rhs=brow_sb[:],
                         start=True, stop=True)
        brep_sb = cp.tile([128, GROUPW * HD], F32)
        for wl in range(GROUPW):
            nc.vector.tensor_copy(out=brep_sb[:, wl * HD:(wl + 1) * HD], in_=brep_ps[:])

        iota_f = cp.tile([128, CT * WIN], BF16)
        nc.sync.dma_start(out=iota_f[:], in_=iota_t[:])
        ident_sb = cp.tile([128, 128], BF16)
        nc.sync.dma_start(out=ident_sb[:], in_=ident_t[:])
        dofc_sb = cp.tile([128, NT], BF16)
        nc.sync.dma_start(out=dofc_sb[:], in_=dofc_t[:])
        aidx_sb = cp.tile([128, NT * 8], I16)
        nc.sync.dma_start(out=aidx_sb[:], in_=Aidx_t[:])

        # er table for own dst range: window w's 128 nodes down partitions
        er_sb = cp.tile([128, NW * H], BF16)
        nc.vector.memset(er_sb[:], 0.0)

        # ---------- phase 1: node row table (bf16) + own-range er ----------
        ntiles = -(-N // 128)
        for b0 in range(0, ntiles, NB):
            bt = min(NB, ntiles - b0)
            n0 = b0 * 128
            bcnt = min(NB * 128, N - n0)
            xt = p1.tile([128, KI, NB * 128], BF16, tag="xt")
            for k in range(KI):
                nc.sync.dma_start(out=xt[:, k, :bcnt],
                                  in_=xT[k * 128:(k + 1) * 128, n0:n0 + bcnt])
            rsb = p1.tile([128, NB, ROWW], BF16, tag="rsb")
            nc.gpsimd.memset(rsb[:, :, MAINW:], 0)
            for j0 in range(0, bt, SB):
                sb = min(SB, bt - j0)
                rps = p1ps.tile([128, SB, AUXW], F32, tag="rps")
                for jj in range(sb):
                    j = j0 + jj
                    cnt = min(128, N - (b0 + j) * 128)
                    for k in range(KI):
                        nc.tensor.matmul(
                            out=rps[:cnt, jj, :],
                            lhsT=xt[:, k, j * 128:j * 128 + cnt],
                            rhs=aux_bf[:, k, :],
                            start=(k == 0), stop=(k == KI - 1))
                # main row payload: [el|g] = aux cols H..AUXW (Scalar engine)
                nc.scalar.activation(out=rsb[:, j0:j0 + sb, :MAINW],
                                     in_=rps[:, :sb, H:AUXW], func=ACTF.Copy)
                # own-range er extraction (rotated: own dst range = tiles 0..NW-1)
                t0 = b0 + j0
                if t0 < NW:
                    nt_er = min(sb, NW - t0)
                    nc.scalar.activation(
                        out=er_sb[:, t0 * H:(t0 + nt_er) * H]
                            .rearrange("p (w h) -> p w h", h=H),
                        in_=rps[:, :nt_er, 0:H], func=ACTF.Copy)
            if bcnt == bt * 128:
                out_ap = row_table[n0:n0 + bt * 128, :].rearrange(
                    "(j p) c -> p j c", p=128)
                nc.sync.dma_start(out=out_ap, in_=rsb[:, :bt, :])
            else:
                full = bcnt // 128
                if full:
                    out_ap = row_table[n0:n0 + full * 128, :].rearrange(
                        "(j p) c -> p j c", p=128)
                    nc.sync.dma_start(out=out_ap, in_=rsb[:, :full, :])
                for j in range(full, bt):
                    cnt = min(128, N - (b0 + j) * 128)
                    nc.sync.dma_start(
                        out=row_table[(b0 + j) * 128:(b0 + j) * 128 + cnt, :],
                        in_=rsb[:cnt, j, :])

        # ---------- phase 2: edge stream ----------
        cur_g = [-1]
        gps_ref = [None]

        def close_group(g):
            gps = gps_ref[0]
            gv = gps[:].rearrange("p (w c) -> p w c", c=MAINW)
            sg = op.tile([128, GROUPW * H], F32, tag="sg")
            nc.vector.tensor_scalar_max(out=sg[:], in0=gv[:, :, 0:H],
                                        scalar1=1e-30)
            rs = op.tile([128, GROUPW * H], F32, tag="rs")
            nc.vector.reciprocal(out=rs[:], in_=sg[:])
            ysb = op.tile([128, GROUPW * HD], F32, tag="ysb")
            nc.vector.tensor_tensor(
                out=ysb[:].rearrange("p (w h d) -> p w h d", h=H, d=D),
                in0=gv[:, :, H:MAINW].rearrange("p w (h d) -> p w h d", h=H),
                in1=rs[:].rearrange("p (w h) -> p w h", h=H)
                    .to_broadcast([128, GROUPW, H, D]),
                op=ALU.mult)
            nc.vector.tensor_add(out=ysb[:], in0=ysb[:], in1=brep_sb[:])
            n0 = g * GROUPW * WIN
            out_ap = y_t[n0:n0 + GROUPW * WIN, :].rearrange(
                "(w p) c -> p w c", p=128)
            nc.sync.dma_start(out=out_ap, in_=ysb[:].rearrange(
                "p (w c) -> p w c", c=HD))

        nreg_cache = {}

        def nreg(n):
            if n not in nreg_cache:
                nreg_cache[n] = nc.gpsimd.to_reg(n)
            return nreg_cache[n]

        gq = [0]  # rotating SWDGE queue counter

        for (c0, ctiles, half) in st.call_meta:
            ne = ctiles * 128
            abuf = gp.tile([128, CT, ROWW], BF16, tag="abuf")
            tab = row_table[half * cfg.SPLIT:(half + 1) * cfg.SPLIT, :]
            nc.gpsimd.dma_gather(abuf[:, :ctiles, :], tab,
                                 aidx_sb[:, c0 * 8:(c0 + ctiles) * 8],
                                 ctiles * 128, nreg(ctiles * 128), ROWW,
                                 queue_num=gq[0] % NQUEUES,
                                 single_packet=False)
            gq[0] += 1

            # batched one-hot S for this call (bf16)
            S_all = tp.tile([128, CT, WIN], BF16, tag="S")
            nc.vector.tensor_tensor(
                out=S_all[:, :ctiles, :],
                in0=dofc_sb[:, c0:c0 + ctiles]
                    .rearrange("p (t o) -> p t o", o=1)
                    .to_broadcast([128, ctiles, WIN]),
                in1=iota_f[:, :ctiles * WIN].rearrange("p (t w) -> p t w", w=WIN),
                op=ALU.is_equal)

            # quad-batched: ST = S^T via PE into one PSUM bank, one Scalar
            # cast per quad, then per-tile er fetch matmuls
            er_ps = erps.tile([128, CT * H], F32, tag="erps")
            for q0 in range(0, ctiles, 4):
                qn = min(4, ctiles - q0)
                st_ps = stpsp.tile([128, 4, 128], F32, tag="stps")
                for jj in range(qn):
                    nc.tensor.matmul(out=st_ps[:, jj, :],
                                     lhsT=S_all[:, q0 + jj, :],
                                     rhs=ident_sb[:], start=True, stop=True)
                st_sb = tp.tile([128, 4, 128], BF16, tag="stsb")
                nc.scalar.activation(out=st_sb[:, :qn, :], in_=st_ps[:, :qn, :],
                                     func=ACTF.Copy)
                for jj in range(qn):
                    j = q0 + jj
                    wv = st.tile_meta[c0 + j][0]
                    nc.tensor.matmul(out=er_ps[:, j * H:(j + 1) * H],
                                     lhsT=st_sb[:, jj, :],
                                     rhs=er_sb[:, wv * H:(wv + 1) * H],
                                     start=True, stop=True)

            # logits -> lrelu -> exp -> mgc (batched)
            esb = tp.tile([128, CT * H], BF16, tag="esb")
            nc.vector.tensor_tensor(
                out=esb[:, :ctiles * H].rearrange("p (t h) -> p t h", h=H),
                in0=abuf[:, :ctiles, 0:H],
                in1=er_ps[:, :ctiles * H].rearrange("p (t h) -> p t h", h=H),
                op=ALU.add)
            nc.vector.scalar_tensor_tensor(
                out=esb[:, :ctiles * H], in0=esb[:, :ctiles * H],
                scalar=NEG_SLOPE, in1=esb[:, :ctiles * H],
                op0=ALU.mult, op1=ALU.max)
            mgc = tp.tile([128, CT, MAINW], BF16, tag="mgc")
            nc.scalar.activation(out=mgc[:, :ctiles, 0:H],
                                 in_=esb[:, :ctiles * H]
                                 .rearrange("p (t h) -> p t h", h=H),
                                 func=ACTF.Exp)
            nc.vector.tensor_tensor(
                out=mgc[:, :ctiles, H:MAINW].rearrange("p t (h d) -> p t h d", h=H),
                in0=abuf[:, :ctiles, H:MAINW].rearrange("p t (h d) -> p t h d", h=H),
                in1=mgc[:, :ctiles, 0:H].to_broadcast([128, ctiles, H, D]),
                op=ALU.mult)

            # per-tile scatter matmuls into window-group accumulators
            for j in range(ctiles):
                tg = c0 + j
                wv, half_, first, last = st.tile_meta[tg]
                g = wv // GROUPW
                if g != cur_g[0]:
                    if cur_g[0] >= 0:
                        close_group(cur_g[0])
                    gps_ref[0] = accp.tile([128, GROUPW * MAINW], F32, tag="gps",
                                           name="gps")
                    cur_g[0] = g
                gps = gps_ref[0]
                wloc = wv - g * GROUPW
                base = wloc * MAINW
                nc.tensor.matmul(out=gps[:, base:base + MAINW],
                                 lhsT=S_all[:, j, :], rhs=mgc[:, j, :],
                                 start=first, stop=last)
        close_group(cur_g[0])

    nc.compile()
    return nc


def make_in_maps(cfg, st, inputs, A_idx, doff):
    import ml_dtypes
    bf = ml_dtypes.bfloat16
    x = np.asarray(inputs["x"], np.float32)
    xT, WT, wcat, bias_flat, fcb_tiled = host_layouts(
        cfg, x, inputs["W"], inputs["attn_l"], inputs["attn_r"],
        inputs["bias"], inputs["fc_w"], inputs["fc_b"])

    def wrap16(a):  # [NPOS] -> [128, NPOS//16]
        return np.tile(np.ascontiguousarray(a.reshape(-1, 16).T), (8, 1))

    in_maps = []
    for c in range(cfg.NCORES):
        dof = doff[c]
        in_maps.append({
            "xT": np.ascontiguousarray(np.roll(xT, -c * cfg.NPC, axis=1)),
            "WT": WT, "wcat": wcat, "bias_flat": bias_flat,
            "fcb_tiled": fcb_tiled,
            "A_idx": wrap16(A_idx[c]),
            "dofc": np.ascontiguousarray(dof.reshape(-1, 128).T).astype(bf),
            "iota_rep": np.tile(np.arange(cfg.WIN), (128, cfg.CT)).astype(bf),
            "ident": np.eye(128, dtype=np.float32).astype(bf),
        })
    return in_maps


def kernel(**inputs):
    import numpy as np
    from concourse import bass_utils

    cfg = Cfg()
    src = np.asarray(inputs["src"])
    dst = np.asarray(inputs["dst"])
    assert src.shape == (cfg.E,) and dst.shape == (cfg.E,)
    st, A_idx, doff = preprocess(cfg, src, dst)
    nc = build_program(cfg, st)
    in_maps = make_in_maps(cfg, st, inputs, A_idx, doff)
    res = bass_utils.run_bass_kernel_spmd(
        nc, in_maps, core_ids=list(range(cfg.NCORES)))
    y = np.concatenate([r["y"][:cfg.NPC] for r in res.results], axis=0)
    return np.ascontiguousarray(y.reshape(cfg.N, cfg.H, cfg.D).astype(np.float32))


# revision 20
# speedup vs baseline: 1.0960x; 1.0411x over previous
"""GAT (graph attention) Bass kernel for TRN2, 8-core SPMD — v3.

Math (equivalent to reference up to fp reassociation):
  feat = x @ W (per head);  el = x @ wl, er = x @ wr (per head)
  g    = feat @ fc_w (per head)
  w_e  = exp(leakyrelu(el[src] + er[dst]))   (softmax without max-subtraction)
  s[d] = sum_{e->d} w_e ;  usum[d] = sum_{e->d} w_e * g[src]
  y[d] = usum[d]/s[d] + bias@fc_w + fc_b

v3 changes vs v2 (963994ns):
  - per-core rotated node table (own dst range first): the own-range er
    values are extracted from the main table pass (no separate er pass /
    xTown input); the rotation is folded into the host-side src remap.
  - dofr (replicated dst-offset rows, 46MB HBM/core) eliminated: only the
    S one-hot is built by is_eq; ST comes from a PE transpose of S
    (identity-rhs matmul) with the PSUM->SBUF cast on the idle Scalar
    engine.
  - DMA batching: 32-tile x loads (1MB each), a single up-front A_idx
    load, one row_table store per batch, one y store per window group
    (y padded to NG*GROUPW*WIN rows, host trims) — cuts the sync
    sequencer's serial DMA dispatch cost ~9x.
  - phase-1 PSUM->SBUF row casts moved to the Scalar engine (batched 4
    tiles per op); rsb pad memset moved to GpSimd (idle in phase 1).

Sharding: dst-range partitioning. Core k owns dst nodes
[k*NPC, (k+1)*NPC). Each core computes the bf16 node row table [el|g] for
all N nodes (rows padded to 256B) in rotated order (own range first),
gathers src rows per edge with dma_gather, and scatter-adds via one-hot
matmuls into PSUM window accumulators (128 dst nodes per window, GROUPW
windows per PSUM bank).

Edges (host-side index prep only) are bucketed by (core, window, src-half);
the src-half split keeps gather indices < 32768 (int16 limit).
"""

import numpy as np

import concourse.bass as bass
import concourse.mybir as mybir
import concourse.tile as tile
from concourse import bacc, library_config

F32 = mybir.dt.float32
BF16 = mybir.dt.bfloat16
I16 = mybir.dt.int16
ALU = mybir.AluOpType
ACTF = mybir.ActivationFunctionType

NEG_SLOPE = 0.2
NQUEUES = 4  # SWDGE queues for gather desc-gen


class Cfg:
    def __init__(self, N=50000, E=1200000, CT=15, GROUPW=10):
        self.N = N
        self.E = E
        self.IN = 256           # input feature dim
        self.H = 4              # heads
        self.O = 64             # per-head out dim
        self.D = 10             # final fc dim
        self.NCORES = 8
        self.WIN = 128          # dst nodes per window
        self.GROUPW = GROUPW    # windows per PSUM accumulator tile
        self.CT = CT            # tiles per gather call
        assert CT * 8 + 1 <= 256  # SWDGE ring: descs per lane per call
        assert N % self.NCORES == 0
        self.NPC = N // self.NCORES
        self.NG = -(-(-(-self.NPC // self.WIN)) // GROUPW)
        self.NW = self.NG * GROUPW  # padded to full groups (empty tail ok)
        assert N % 2 == 0
        self.SPLIT = N // 2
        assert self.SPLIT <= 32767 and self.NPC <= 32767
        self.KI = -(-self.IN // 128)           # input-dim chunks (2)
        self.KHO = -(-(self.H * self.O) // 128)  # head*out chunks (2)
        self.AUXW = 2 * self.H + self.H * self.D  # 48: [er|el|g]
        self.ROWW = 128                         # bf16 row width = 256B
        self.HD = self.H * self.D               # 40
        self.MAINW = self.H + self.HD           # 44: [el|g] payload width
        self.NB = 32            # node tiles per phase-1 load batch
        self.SB = 4             # tiles per phase-1 PSUM sub-batch
        self.NYR = self.NG * GROUPW * self.WIN  # padded y rows per core


class Structure:
    """Compile-time stream structure shared by host packer and program builder."""

    def __init__(self, cfg: Cfg, T_wh: np.ndarray):
        self.T_wh = T_wh  # [NW, 2] tiles per (window, half)
        # half-major order: all groups' half-0 tiles first, then half-1 —
        # half-0 gathers overlap the half-1 table build. Each (group, half)
        # segment accumulates separately (h0 partials staged to SBUF).
        tile_meta = []    # (w, half, seg_first, seg_last)
        call_meta = []    # (tile_start, ntiles, half)
        pos_wh = np.zeros((cfg.NW, 2), np.int64)
        for half in (0, 1):
            for g in range(cfg.NG):
                ws = range(g * cfg.GROUPW, min((g + 1) * cfg.GROUPW, cfg.NW))
                seg_start = len(tile_meta)
                seg_n = int(T_wh[list(ws), half].sum())
                for w in ws:
                    pos_wh[w, half] = len(tile_meta)
                    for t in range(T_wh[w, half]):
                        first = (len(tile_meta) == seg_start)
                        last = (len(tile_meta) == seg_start + seg_n - 1)
                        tile_meta.append((w, half, first, last))
                o = 0
                while o < seg_n:
                    c = min(cfg.CT, seg_n - o)
                    call_meta.append((seg_start + o, c, half))
                    o += c
        self.tile_meta = tile_meta
        self.call_meta = call_meta
        self.pos_wh = pos_wh
        self.NT = len(tile_meta)


def preprocess(cfg: Cfg, src: np.ndarray, dst: np.ndarray):
    """Host-side index/layout prep (no float arithmetic).

    Returns (structure, A_idx[NC,NPOS] i16, doff[NC,NPOS] f32)."""
    src = np.asarray(src, np.int64)
    dst = np.asarray(dst, np.int64)
    core = dst // cfg.NPC
    dstl = dst - core * cfg.NPC
    w = dstl // cfg.WIN
    off = dstl - w * cfg.WIN
    # per-core rotation: core c's table row r holds node (c*NPC + r) mod N
    srcr_all = (src - core * cfg.NPC) % cfg.N
    half = (srcr_all >= cfg.SPLIT).astype(np.int64)
    srcr = srcr_all - half * cfg.SPLIT

    key = (core * cfg.NW + w) * 2 + half
    order = np.argsort(key, kind="stable")
    nk = cfg.NCORES * cfg.NW * 2
    cnt = np.bincount(key, minlength=nk)
    starts = np.zeros(nk + 1, np.int64)
    np.cumsum(cnt, out=starts[1:])
    cnt_cwh = cnt.reshape(cfg.NCORES, cfg.NW, 2)
    T_wh = np.maximum(-(-cnt_cwh.max(axis=0) // cfg.WIN), 1)  # [NW,2]
    st = Structure(cfg, T_wh)

    NPOS = st.NT * 128
    A_idx = np.zeros((cfg.NCORES, NPOS), np.int16)
    doff = np.full((cfg.NCORES, NPOS), -1.0, np.float32)
    for c in range(cfg.NCORES):
        for wv in range(cfg.NW):
            for h in (0, 1):
                kk = (c * cfg.NW + wv) * 2 + h
                n = cnt[kk]
                if n == 0:
                    continue
                eids = order[starts[kk]:starts[kk] + n]
                p0 = st.pos_wh[wv, h] * 128
                A_idx[c, p0:p0 + n] = srcr[eids]
                doff[c, p0:p0 + n] = off[eids]
    return st, A_idx, doff


def host_layouts(cfg: Cfg, x, W, attn_l, attn_r, bias, fc_w, fc_b):
    """Pure layout transforms of the inputs (no model arithmetic)."""
    import ml_dtypes
    bf = ml_dtypes.bfloat16
    H, O, D, IN = cfg.H, cfg.O, cfg.D, cfg.IN
    xT = np.ascontiguousarray(np.asarray(x, np.float32).T).astype(bf)  # [IN, N]
    WT = np.ascontiguousarray(
        np.asarray(W, np.float32).transpose(0, 2, 1).reshape(H * O, IN))
    wcat = np.zeros((H * O, cfg.AUXW), np.float32)  # [ho, er|el|g]
    for h in range(H):
        r = slice(h * O, (h + 1) * O)
        wcat[r, h] = attn_r[h]
        wcat[r, H + h] = attn_l[h]
        wcat[r, 2 * H + h * D:2 * H + (h + 1) * D] = fc_w
    bias_flat = np.asarray(bias, np.float32).reshape(H * O, 1)
    fcb_tiled = np.tile(np.asarray(fc_b, np.float32), H).reshape(1, H * D)
    return xT, WT, wcat, bias_flat, fcb_tiled


def build_program(cfg: Cfg, st: Structure):
    nc = bacc.Bacc(trn_type="TRN2", num_swdge_queues=NQUEUES,
                   dynamic_dma_scratch_size=16384)
    N, IN, H, O, D = cfg.N, cfg.IN, cfg.H, cfg.O, cfg.D
    KI, KHO, AUXW, ROWW, HD, MAINW = (cfg.KI, cfg.KHO, cfg.AUXW, cfg.ROWW,
                                      cfg.HD, cfg.MAINW)
    WIN, NW, NG, GROUPW, NPC, CT = (cfg.WIN, cfg.NW, cfg.NG, cfg.GROUPW,
                                    cfg.NPC, cfg.CT)
    NB, SB = cfg.NB, cfg.SB
    NT = st.NT

    xT = nc.dram_tensor("xT", [IN, N], BF16, kind="ExternalInput")
    WTt = nc.dram_tensor("WT", [H * O, IN], F32, kind="ExternalInput")
    wcat_t = nc.dram_tensor("wcat", [H * O, AUXW], F32, kind="ExternalInput")
    bias_t = nc.dram_tensor("bias_flat", [H * O, 1], F32, kind="ExternalInput")
    fcb_t = nc.dram_tensor("fcb_tiled", [1, HD], F32, kind="ExternalInput")
    Aidx_t = nc.dram_tensor("A_idx", [128, NT * 8], I16, kind="ExternalInput")
    dofc_t = nc.dram_tensor("dofc", [128, NT], BF16, kind="ExternalInput")
    iota_t = nc.dram_tensor("iota_rep", [128, CT * WIN], BF16,
                            kind="ExternalInput")
    ident_t = nc.dram_tensor("ident", [128, 128], BF16, kind="ExternalInput")
    y_t = nc.dram_tensor("y", [cfg.NYR, HD], F32, kind="ExternalOutput")

    row_table = nc.dram_tensor("row_table", [N, ROWW], BF16, kind="Internal")

    with tile.TileContext(nc) as tc, \
            tc.tile_pool(name="const", bufs=1) as cp, \
            tc.tile_pool(name="p1", bufs=2) as p1, \
            tc.tile_pool(name="p1ps", bufs=2, space="PSUM") as p1ps, \
            tc.tile_pool(name="stps", bufs=2, space="PSUM") as stpsp, \
            tc.tile_pool(name="gath", bufs=8) as gp, \
            tc.tile_pool(name="tp", bufs=2) as tp, \
            tc.tile_pool(name="erps", bufs=2, space="PSUM") as erps, \
            tc.tile_pool(name="acc", bufs=2, space="PSUM") as accp, \
            tc.tile_pool(name="outp", bufs=2) as op:

        # ---------- phase 0: constants ----------
        wt_sb = cp.tile([128, KHO, IN], F32)
        wcat_sb = cp.tile([128, KHO, AUXW], F32)
        bf_sb = cp.tile([128, KHO, 1], F32)
        for a in range(KHO):
            r = slice(a * 128, (a + 1) * 128)
            nc.sync.dma_start(out=wt_sb[:, a, :], in_=WTt[r, :])
            nc.sync.dma_start(out=wcat_sb[:, a, :], in_=wcat_t[r, :])
            nc.sync.dma_start(out=bf_sb[:, a, :], in_=bias_t[r, :])
        fcb_sb = cp.tile([1, HD], F32)
        nc.sync.dma_start(out=fcb_sb[:], in_=fcb_t[:])

        # aux projection matrix [IN, er|el|g] = WT.T @ wcat, then bf16
        aux_bf = cp.tile([128, KI, AUXW], BF16)
        for m in range(KI):
            aps = p1ps.tile([128, AUXW], F32, tag="rps")
            for k in range(KHO):
                nc.tensor.matmul(out=aps[:], lhsT=wt_sb[:, k, m * 128:(m + 1) * 128],
                                 rhs=wcat_sb[:, k, :], start=(k == 0), stop=(k == KHO - 1))
            nc.vector.tensor_copy(out=aux_bf[:, m, :], in_=aps[:])

        # bias@fc_w + fc_b, replicated to 128 partitions and GROUPW windows
        brow_ps = p1ps.tile([1, HD], F32, tag="rps")
        for k in range(KHO):
            nc.tensor.matmul(out=brow_ps[:], lhsT=bf_sb[:, k, :],
                             rhs=wcat_sb[:, k, 2 * H:AUXW],
                             start=(k == 0), stop=(k == KHO - 1))
        brow_sb = cp.tile([1, HD], F32)
        nc.vector.tensor_add(out=brow_sb[:], in0=brow_ps[:], in1=fcb_sb[:])
        ones_sb = cp.tile([1, 128], F32)
        nc.vector.memset(ones_sb[:], 1.0)
        brep_ps = p1ps.tile([128, HD], F32, tag="rps")
        nc.tensor.matmul(out=brep_ps[:], lhsT=ones_sb[:], rhs=brow_sb[:],
                         start=True, stop=True)
        brep_sb = cp.tile([128, GROUPW * HD], F32)
        for wl in range(GROUPW):
            nc.vector.tensor_copy(out=brep_sb[:, wl * HD:(wl + 1) * HD], in_=brep_ps[:])

        iota_f = cp.tile([128, CT * WIN], BF16)
        nc.sync.dma_start(out=iota_f[:], in_=iota_t[:])
        ident_sb = cp.tile([128, 128], BF16)
        nc.sync.dma_start(out=ident_sb[:], in_=ident_t[:])
        dofc_sb = cp.tile([128, NT], BF16)
        nc.sync.dma_start(out=dofc_sb[:], in_=dofc_t[:])
        aidx_sb = cp.tile([128, NT * 8], I16)
        nc.sync.dma_start(out=aidx_sb[:], in_=Aidx_t[:])

        # er table for own dst range: window w's 128 nodes down partitions
        er_sb = cp.tile([128, NW * H], BF16)
        nc.vector.memset(er_sb[:], 0.0)

        # ---------- phase 1: node row table (bf16) + own-range er ----------
        ntiles = -(-N // 128)
        for b0 in range(0, ntiles, NB):
            bt = min(NB, ntiles - b0)
            n0 = b0 * 128
            bcnt = min(NB * 128, N - n0)
            xt = p1.tile([128, KI, NB * 128], BF16, tag="xt")
            for k in range(KI):
                nc.sync.dma_start(out=xt[:, k, :bcnt],
                                  in_=xT[k * 128:(k + 1) * 128, n0:n0 + bcnt])
            rsb = p1.tile([128, NB, ROWW], BF16, tag="rsb")
            nc.gpsimd.memset(rsb[:, :, MAINW:], 0)
            for j0 in range(0, bt, SB):
                sb = min(SB, bt - j0)
                rps = p1ps.tile([128, SB, AUXW], F32, tag="rps")
                for jj in range(sb):
                    j = j0 + jj
                    cnt = min(128, N - (b0 + j) * 128)
                    for k in range(KI):
                        nc.tensor.matmul(
                            out=rps[:cnt, jj, :],
                            lhsT=xt[:, k, j * 128:j * 128 + cnt],
                            rhs=aux_bf[:, k, :],
                            start=(k == 0), stop=(k == KI - 1))
                # main row payload: [el|g] = aux cols H..AUXW (Scalar engine)
                nc.scalar.activation(out=rsb[:, j0:j0 + sb, :MAINW],
                                     in_=rps[:, :sb, H:AUXW], func=ACTF.Copy)
                # own-range er extraction (rotated: own dst range = tiles 0..NW-1)
                t0 = b0 + j0
                if t0 < NW:
                    nt_er = min(sb, NW - t0)
                    nc.scalar.activation(
                        out=er_sb[:, t0 * H:(t0 + nt_er) * H]
                            .rearrange("p (w h) -> p w h", h=H),
                        in_=rps[:, :nt_er, 0:H], func=ACTF.Copy)
            if bcnt == bt * 128:
                out_ap = row_table[n0:n0 + bt * 128, :].rearrange(
                    "(j p) c -> p j c", p=128)
                nc.sync.dma_start(out=out_ap, in_=rsb[:, :bt, :])
            else:
                full = bcnt // 128
                if full:
                    out_ap = row_table[n0:n0 + full * 128, :].rearrange(
                        "(j p) c -> p j c", p=128)
                    nc.sync.dma_start(out=out_ap, in_=rsb[:, :full, :])
                for j in range(full, bt):
                    cnt = min(128, N - (b0 + j) * 128)
                    nc.sync.dma_start(
                        out=row_table[(b0 + j) * 128:(b0 + j) * 128 + cnt, :],
                        in_=rsb[:cnt, j, :])

        # ---------- phase 2: edge stream ----------
        cur_seg = [None]  # (group, half)
        gps_ref = [None]
        stage_ref = [None] * NG

        def stage_group(g):
            # h0 segment done: park partial sums in SBUF, free the PSUM bank
            stg = op.tile([128, GROUPW * MAINW], F32, tag="stage", bufs=NG,
                          name=f"stage{g}")
            stage_ref[g] = stg
            nc.scalar.activation(out=stg[:], in_=gps_ref[0][:], func=ACTF.Copy)

        def close_group(g):
            tot = op.tile([128, GROUPW * MAINW], F32, tag="tot")
            nc.vector.tensor_add(out=tot[:], in0=gps_ref[0][:],
                                 in1=stage_ref[g][:])
            gv = tot[:].rearrange("p (w c) -> p w c", c=MAINW)
            sg = op.tile([128, GROUPW * H], F32, tag="sg")
            nc.vector.tensor_scalar_max(out=sg[:], in0=gv[:, :, 0:H],
                                        scalar1=1e-30)
            rs = op.tile([128, GROUPW * H], F32, tag="rs")
            nc.vector.reciprocal(out=rs[:], in_=sg[:])
            ysb = op.tile([128, GROUPW * HD], F32, tag="ysb")
            nc.vector.tensor_tensor(
                out=ysb[:].rearrange("p (w h d) -> p w h d", h=H, d=D),
                in0=gv[:, :, H:MAINW].rearrange("p w (h d) -> p w h d", h=H),
                in1=rs[:].rearrange("p (w h) -> p w h", h=H)
                    .to_broadcast([128, GROUPW, H, D]),
                op=ALU.mult)
            nc.vector.tensor_add(out=ysb[:], in0=ysb[:], in1=brep_sb[:])
            n0 = g * GROUPW * WIN
            out_ap = y_t[n0:n0 + GROUPW * WIN, :].rearrange(
                "(w p) c -> p w c", p=128)
            nc.sync.dma_start(out=out_ap, in_=ysb[:].rearrange(
                "p (w c) -> p w c", c=HD))

        def end_segment(seg):
            if seg is None:
                return
            g, h = seg
            if h == 0:
                stage_group(g)
            else:
                close_group(g)

        nreg_cache = {}

        def nreg(n):
            if n not in nreg_cache:
                nreg_cache[n] = nc.gpsimd.to_reg(n)
            return nreg_cache[n]

        gq = [0]  # rotating SWDGE queue counter

        for (c0, ctiles, half) in st.call_meta:
            ne = ctiles * 128
            abuf = gp.tile([128, CT, ROWW], BF16, tag="abuf")
            tab = row_table[half * cfg.SPLIT:(half + 1) * cfg.SPLIT, :]
            nc.gpsimd.dma_gather(abuf[:, :ctiles, :], tab,
                                 aidx_sb[:, c0 * 8:(c0 + ctiles) * 8],
                                 ctiles * 128, nreg(ctiles * 128), ROWW,
                                 queue_num=gq[0] % NQUEUES,
                                 single_packet=False)
            gq[0] += 1

            # batched one-hot S for this call (bf16)
            S_all = tp.tile([128, CT, WIN], BF16, tag="S")
            nc.vector.tensor_tensor(
                out=S_all[:, :ctiles, :],
                in0=dofc_sb[:, c0:c0 + ctiles]
                    .rearrange("p (t o) -> p t o", o=1)
                    .to_broadcast([128, ctiles, WIN]),
                in1=iota_f[:, :ctiles * WIN].rearrange("p (t w) -> p t w", w=WIN),
                op=ALU.is_equal)

            # quad-batched: ST = S^T via PE into one PSUM bank, one Scalar
            # cast per quad, then per-tile er fetch matmuls
            er_ps = erps.tile([128, CT * H], F32, tag="erps")
            for q0 in range(0, ctiles, 4):
                qn = min(4, ctiles - q0)
                st_ps = stpsp.tile([128, 4, 128], F32, tag="stps")
                for jj in range(qn):
                    nc.tensor.matmul(out=st_ps[:, jj, :],
                                     lhsT=S_all[:, q0 + jj, :],
                                     rhs=ident_sb[:], start=True, stop=True)
                st_sb = tp.tile([128, 4, 128], BF16, tag="stsb")
                nc.scalar.activation(out=st_sb[:, :qn, :], in_=st_ps[:, :qn, :],
                                     func=ACTF.Copy)
                for jj in range(qn):
                    j = q0 + jj
                    wv = st.tile_meta[c0 + j][0]
                    nc.tensor.matmul(out=er_ps[:, j * H:(j + 1) * H],
                                     lhsT=st_sb[:, jj, :],
                                     rhs=er_sb[:, wv * H:(wv + 1) * H],
                                     start=True, stop=True)

            # logits -> lrelu -> exp -> mgc (batched)
            esb = tp.tile([128, CT * H], BF16, tag="esb")
            nc.vector.tensor_tensor(
                out=esb[:, :ctiles * H].rearrange("p (t h) -> p t h", h=H),
                in0=abuf[:, :ctiles, 0:H],
                in1=er_ps[:, :ctiles * H].rearrange("p (t h) -> p t h", h=H),
                op=ALU.add)
            nc.vector.scalar_tensor_tensor(
                out=esb[:, :ctiles * H], in0=esb[:, :ctiles * H],
                scalar=NEG_SLOPE, in1=esb[:, :ctiles * H],
                op0=ALU.mult, op1=ALU.max)
            mgc = tp.tile([128, CT, MAINW], BF16, tag="mgc")
            nc.scalar.activation(out=mgc[:, :ctiles, 0:H],
                                 in_=esb[:, :ctiles * H]
                                 .rearrange("p (t h) -> p t h", h=H),
                                 func=ACTF.Exp)
            nc.vector.tensor_tensor(
                out=mgc[:, :ctiles, H:MAINW].rearrange("p t (h d) -> p t h d", h=H),
                in0=abuf[:, :ctiles, H:MAINW].rearrange("p t (h d) -> p t h d", h=H),
                in1=mgc[:, :ctiles, 0:H].to_broadcast([128, ctiles, H, D]),
                op=ALU.mult)

            # per-tile scatter matmuls into window-group accumulators
            for j in range(ctiles):
                tg = c0 + j
                wv, half_, first, last = st.tile_meta[tg]
                g = wv // GROUPW
                seg = (g, half_)
                if seg != cur_seg[0]:
                    end_segment(cur_seg[0])
                    gps_ref[0] = accp.tile([128, GROUPW * MAINW], F32, tag="gps",
                                           name="gps")
                    cur_seg[0] = seg
                gps = gps_ref[0]
                wloc = wv - g * GROUPW
                base = wloc * MAINW
                nc.tensor.matmul(out=gps[:, base:base + MAINW],
                                 lhsT=S_all[:, j, :], rhs=mgc[:, j, :],
                                 start=first, stop=last)
        end_segment(cur_seg[0])

    nc.compile()
    return nc


def make_in_maps(cfg, st, inputs, A_idx, doff):
    import ml_dtypes
    bf = ml_dtypes.bfloat16
    x = np.asarray(inputs["x"], np.float32)
    xT, WT, wcat, bias_flat, fcb_tiled = host_layouts(
        cfg, x, inputs["W"], inputs["attn_l"], inputs["attn_r"],
        inputs["bias"], inputs["fc_w"], inputs["fc_b"])

    def wrap16(a):  # [NPOS] -> [128, NPOS//16]
        return np.tile(np.ascontiguousarray(a.reshape(-1, 16).T), (8, 1))

    in_maps = []
    for c in range(cfg.NCORES):
        dof = doff[c]
        in_maps.append({
            "xT": np.ascontiguousarray(np.roll(xT, -c * cfg.NPC, axis=1)),
            "WT": WT, "wcat": wcat, "bias_flat": bias_flat,
            "fcb_tiled": fcb_tiled,
            "A_idx": wrap16(A_idx[c]),
            "dofc": np.ascontiguousarray(dof.reshape(-1, 128).T).astype(bf),
            "iota_rep": np.tile(np.arange(cfg.WIN), (128, cfg.CT)).astype(bf),
            "ident": np.eye(128, dtype=np.float32).astype(bf),
        })
    return in_maps


def kernel(**inputs):
    import numpy as np
    from concourse import bass_utils

    cfg = Cfg()
    src = np.asarray(inputs["src"])
    dst = np.asarray(inputs["dst"])
    assert src.shape == (cfg.E,) and dst.shape == (cfg.E,)
    st, A_idx, doff = preprocess(cfg, src, dst)
    nc = build_program(cfg, st)
    in_maps = make_in_maps(cfg, st, inputs, A_idx, doff)
    res = bass_utils.run_bass_kernel_spmd(
        nc, in_maps, core_ids=list(range(cfg.NCORES)))
    y = np.concatenate([r["y"][:cfg.NPC] for r in res.results], axis=0)
    return np.ascontiguousarray(y.reshape(cfg.N, cfg.H, cfg.D).astype(np.float32))


# revision 28
# speedup vs baseline: 1.1473x; 1.0468x over previous
"""GAT (graph attention) Bass kernel for TRN2, 8-core SPMD — v3.

Math (equivalent to reference up to fp reassociation):
  feat = x @ W (per head);  el = x @ wl, er = x @ wr (per head)
  g    = feat @ fc_w (per head)
  w_e  = exp(leakyrelu(el[src] + er[dst]))   (softmax without max-subtraction)
  s[d] = sum_{e->d} w_e ;  usum[d] = sum_{e->d} w_e * g[src]
  y[d] = usum[d]/s[d] + bias@fc_w + fc_b

v3 changes vs v2 (963994ns):
  - per-core rotated node table (own dst range first): the own-range er
    values are extracted from the main table pass (no separate er pass /
    xTown input); the rotation is folded into the host-side src remap.
  - dofr (replicated dst-offset rows, 46MB HBM/core) eliminated: only the
    S one-hot is built by is_eq; ST comes from a PE transpose of S
    (identity-rhs matmul) with the PSUM->SBUF cast on the idle Scalar
    engine.
  - DMA batching: 32-tile x loads (1MB each), a single up-front A_idx
    load, one row_table store per batch, one y store per window group
    (y padded to NG*GROUPW*WIN rows, host trims) — cuts the sync
    sequencer's serial DMA dispatch cost ~9x.
  - phase-1 PSUM->SBUF row casts moved to the Scalar engine (batched 4
    tiles per op); rsb pad memset moved to GpSimd (idle in phase 1).

Sharding: dst-range partitioning. Core k owns dst nodes
[k*NPC, (k+1)*NPC). Each core computes the bf16 node row table [el|g] for
all N nodes (rows padded to 256B) in rotated order (own range first),
gathers src rows per edge with dma_gather, and scatter-adds via one-hot
matmuls into PSUM window accumulators (128 dst nodes per window, GROUPW
windows per PSUM bank).

Edges (host-side index prep only) are bucketed by (core, window, src-half);
the src-half split keeps gather indices < 32768 (int16 limit).
"""

import numpy as np

import concourse.bass as bass
import concourse.mybir as mybir
import concourse.tile as tile
from concourse import bacc, library_config

F32 = mybir.dt.float32
BF16 = mybir.dt.bfloat16
I16 = mybir.dt.int16
ALU = mybir.AluOpType
ACTF = mybir.ActivationFunctionType

NEG_SLOPE = 0.2
NQUEUES = 4  # SWDGE queues for gather desc-gen


class Cfg:
    def __init__(self, N=50000, E=1200000, CT=15, GROUPW=10):
        self.N = N
        self.E = E
        self.IN = 256           # input feature dim
        self.H = 4              # heads
        self.O = 64             # per-head out dim
        self.D = 10             # final fc dim
        self.NCORES = 8
        self.WIN = 128          # dst nodes per window
        self.GROUPW = GROUPW    # windows per PSUM accumulator tile
        self.CT = CT            # tiles per gather call
        assert CT * 8 + 1 <= 256  # SWDGE ring: descs per lane per call
        assert N % self.NCORES == 0
        self.NPC = N // self.NCORES
        self.NG = -(-(-(-self.NPC // self.WIN)) // GROUPW)
        self.NW = self.NG * GROUPW  # padded to full groups (empty tail ok)
        assert N % 2 == 0
        self.SPLIT = N // 2
        assert self.SPLIT <= 32767 and self.NPC <= 32767
        self.KI = -(-self.IN // 128)           # input-dim chunks (2)
        self.KHO = -(-(self.H * self.O) // 128)  # head*out chunks (2)
        self.AUXW = 2 * self.H + self.H * self.D  # 48: [er|el|g]
        self.ROWW = 128                         # bf16 row width = 256B
        self.HD = self.H * self.D               # 40
        self.MAINW = self.H + self.HD           # 44: [el|g] payload width
        self.NB = 32            # node tiles per phase-1 load batch
        self.SB = 8             # tiles per phase-1 PSUM sub-batch
        self.NYR = self.NG * GROUPW * self.WIN  # padded y rows per core


class Structure:
    """Compile-time stream structure shared by host packer and program builder."""

    def __init__(self, cfg: Cfg, T_wh: np.ndarray):
        self.T_wh = T_wh  # [NW, 2] tiles per (window, half)
        # half-major order: all groups' half-0 tiles first, then half-1 —
        # half-0 gathers overlap the half-1 table build. Each (group, half)
        # segment accumulates separately (h0 partials staged to SBUF).
        tile_meta = []    # (w, half, seg_first, seg_last)
        call_meta = []    # (tile_start, ntiles, half)
        pos_wh = np.zeros((cfg.NW, 2), np.int64)
        for half in (0, 1):
            for g in range(cfg.NG):
                ws = range(g * cfg.GROUPW, min((g + 1) * cfg.GROUPW, cfg.NW))
                seg_start = len(tile_meta)
                seg_n = int(T_wh[list(ws), half].sum())
                for w in ws:
                    pos_wh[w, half] = len(tile_meta)
                    for t in range(T_wh[w, half]):
                        first = (len(tile_meta) == seg_start)
                        last = (len(tile_meta) == seg_start + seg_n - 1)
                        tile_meta.append((w, half, first, last))
                o = 0
                while o < seg_n:
                    c = min(cfg.CT, seg_n - o)
                    call_meta.append((seg_start + o, c, half))
                    o += c
        self.tile_meta = tile_meta
        self.call_meta = call_meta
        self.pos_wh = pos_wh
        self.NT = len(tile_meta)


def preprocess(cfg: Cfg, src: np.ndarray, dst: np.ndarray):
    """Host-side index/layout prep (no float arithmetic).

    Returns (structure, A_idx[NC,NPOS] i16, doff[NC,NPOS] f32)."""
    src = np.asarray(src, np.int64)
    dst = np.asarray(dst, np.int64)
    core = dst // cfg.NPC
    dstl = dst - core * cfg.NPC
    w = dstl // cfg.WIN
    off = dstl - w * cfg.WIN
    # per-core rotation: core c's table row r holds node (c*NPC + r) mod N
    srcr_all = (src - core * cfg.NPC) % cfg.N
    half = (srcr_all >= cfg.SPLIT).astype(np.int64)
    srcr = srcr_all - half * cfg.SPLIT

    key = (core * cfg.NW + w) * 2 + half
    order = np.argsort(key, kind="stable")
    nk = cfg.NCORES * cfg.NW * 2
    cnt = np.bincount(key, minlength=nk)
    starts = np.zeros(nk + 1, np.int64)
    np.cumsum(cnt, out=starts[1:])
    cnt_cwh = cnt.reshape(cfg.NCORES, cfg.NW, 2)
    T_wh = np.maximum(-(-cnt_cwh.max(axis=0) // cfg.WIN), 1)  # [NW,2]
    st = Structure(cfg, T_wh)

    NPOS = st.NT * 128
    A_idx = np.zeros((cfg.NCORES, NPOS), np.int16)
    doff = np.full((cfg.NCORES, NPOS), -1.0, np.float32)
    for c in range(cfg.NCORES):
        for wv in range(cfg.NW):
            for h in (0, 1):
                kk = (c * cfg.NW + wv) * 2 + h
                n = cnt[kk]
                if n == 0:
                    continue
                eids = order[starts[kk]:starts[kk] + n]
                p0 = st.pos_wh[wv, h] * 128
                A_idx[c, p0:p0 + n] = srcr[eids]
                doff[c, p0:p0 + n] = off[eids]
    return st, A_idx, doff


def host_layouts(cfg: Cfg, x, W, attn_l, attn_r, bias, fc_w, fc_b):
    """Pure layout transforms of the inputs (no model arithmetic)."""
    import ml_dtypes
    bf = ml_dtypes.bfloat16
    H, O, D, IN = cfg.H, cfg.O, cfg.D, cfg.IN
    xT = np.ascontiguousarray(np.asarray(x, np.float32).T).astype(bf)  # [IN, N]
    WT = np.ascontiguousarray(
        np.asarray(W, np.float32).transpose(0, 2, 1).reshape(H * O, IN))
    wcat = np.zeros((H * O, cfg.AUXW), np.float32)  # [ho, er|el|g]
    for h in range(H):
        r = slice(h * O, (h + 1) * O)
        wcat[r, h] = attn_r[h]
        wcat[r, H + h] = attn_l[h]
        wcat[r, 2 * H + h * D:2 * H + (h + 1) * D] = fc_w
    bias_flat = np.asarray(bias, np.float32).reshape(H * O, 1)
    fcb_tiled = np.tile(np.asarray(fc_b, np.float32), H).reshape(1, H * D)
    return xT, WT, wcat, bias_flat, fcb_tiled


def build_program(cfg: Cfg, st: Structure):
    nc = bacc.Bacc(trn_type="TRN2", num_swdge_queues=NQUEUES,
                   dynamic_dma_scratch_size=16384)
    N, IN, H, O, D = cfg.N, cfg.IN, cfg.H, cfg.O, cfg.D
    KI, KHO, AUXW, ROWW, HD, MAINW = (cfg.KI, cfg.KHO, cfg.AUXW, cfg.ROWW,
                                      cfg.HD, cfg.MAINW)
    WIN, NW, NG, GROUPW, NPC, CT = (cfg.WIN, cfg.NW, cfg.NG, cfg.GROUPW,
                                    cfg.NPC, cfg.CT)
    NB, SB = cfg.NB, cfg.SB
    NT = st.NT

    xT = nc.dram_tensor("xT", [IN, N], BF16, kind="ExternalInput")
    WTt = nc.dram_tensor("WT", [H * O, IN], F32, kind="ExternalInput")
    wcat_t = nc.dram_tensor("wcat", [H * O, AUXW], F32, kind="ExternalInput")
    bias_t = nc.dram_tensor("bias_flat", [H * O, 1], F32, kind="ExternalInput")
    fcb_t = nc.dram_tensor("fcb_tiled", [1, HD], F32, kind="ExternalInput")
    Aidx_t = nc.dram_tensor("A_idx", [128, NT * 8], I16, kind="ExternalInput")
    dofc_t = nc.dram_tensor("dofc", [128, NT], BF16, kind="ExternalInput")
    iota_t = nc.dram_tensor("iota_rep", [128, CT * WIN], BF16,
                            kind="ExternalInput")
    ident_t = nc.dram_tensor("ident", [128, 128], BF16, kind="ExternalInput")
    y_t = nc.dram_tensor("y", [cfg.NYR, HD], F32, kind="ExternalOutput")

    row_table = nc.dram_tensor("row_table", [N, ROWW], BF16, kind="Internal")

    with tile.TileContext(nc) as tc, \
            tc.tile_pool(name="const", bufs=1) as cp, \
            tc.tile_pool(name="p1", bufs=2) as p1, \
            tc.tile_pool(name="p1ps", bufs=2, space="PSUM") as p1ps, \
            tc.tile_pool(name="stps", bufs=2, space="PSUM") as stpsp, \
            tc.tile_pool(name="gath", bufs=8) as gp, \
            tc.tile_pool(name="tp", bufs=2) as tp, \
            tc.tile_pool(name="erps", bufs=2, space="PSUM") as erps, \
            tc.tile_pool(name="acc", bufs=2, space="PSUM") as accp, \
            tc.tile_pool(name="outp", bufs=2) as op:

        # ---------- phase 0: constants ----------
        wt_sb = cp.tile([128, KHO, IN], F32)
        wcat_sb = cp.tile([128, KHO, AUXW], F32)
        bf_sb = cp.tile([128, KHO, 1], F32)
        for a in range(KHO):
            r = slice(a * 128, (a + 1) * 128)
            nc.sync.dma_start(out=wt_sb[:, a, :], in_=WTt[r, :])
            nc.sync.dma_start(out=wcat_sb[:, a, :], in_=wcat_t[r, :])
            nc.sync.dma_start(out=bf_sb[:, a, :], in_=bias_t[r, :])
        fcb_sb = cp.tile([1, HD], F32)
        nc.sync.dma_start(out=fcb_sb[:], in_=fcb_t[:])

        # aux projection matrix [IN, er|el|g] = WT.T @ wcat, then bf16
        aux_bf = cp.tile([128, KI, AUXW], BF16)
        for m in range(KI):
            aps = p1ps.tile([128, AUXW], F32, tag="rps")
            for k in range(KHO):
                nc.tensor.matmul(out=aps[:], lhsT=wt_sb[:, k, m * 128:(m + 1) * 128],
                                 rhs=wcat_sb[:, k, :], start=(k == 0), stop=(k == KHO - 1))
            nc.vector.tensor_copy(out=aux_bf[:, m, :], in_=aps[:])

        # bias@fc_w + fc_b, replicated to 128 partitions and GROUPW windows
        brow_ps = p1ps.tile([1, HD], F32, tag="rps")
        for k in range(KHO):
            nc.tensor.matmul(out=brow_ps[:], lhsT=bf_sb[:, k, :],
                             rhs=wcat_sb[:, k, 2 * H:AUXW],
                             start=(k == 0), stop=(k == KHO - 1))
        brow_sb = cp.tile([1, HD], F32)
        nc.vector.tensor_add(out=brow_sb[:], in0=brow_ps[:], in1=fcb_sb[:])
        ones_sb = cp.tile([1, 128], F32)
        nc.vector.memset(ones_sb[:], 1.0)
        brep_ps = p1ps.tile([128, HD], F32, tag="rps")
        nc.tensor.matmul(out=brep_ps[:], lhsT=ones_sb[:], rhs=brow_sb[:],
                         start=True, stop=True)
        brep_sb = cp.tile([128, GROUPW * HD], F32)
        for wl in range(GROUPW):
            nc.vector.tensor_copy(out=brep_sb[:, wl * HD:(wl + 1) * HD], in_=brep_ps[:])

        iota_f = cp.tile([128, CT * WIN], BF16)
        nc.sync.dma_start(out=iota_f[:], in_=iota_t[:])
        ident_sb = cp.tile([128, 128], BF16)
        nc.sync.dma_start(out=ident_sb[:], in_=ident_t[:])
        dofc_sb = cp.tile([128, NT], BF16)
        nc.sync.dma_start(out=dofc_sb[:], in_=dofc_t[:])
        aidx_sb = cp.tile([128, NT * 8], I16)
        nc.sync.dma_start(out=aidx_sb[:], in_=Aidx_t[:])

        # er table for own dst range: window w's 128 nodes down partitions
        er_sb = cp.tile([128, NW * H], BF16)
        nc.vector.memset(er_sb[:], 0.0)

        # ---------- phase 1: node row table (bf16) + own-range er ----------
        ntiles = -(-N // 128)
        for b0 in range(0, ntiles, NB):
            bt = min(NB, ntiles - b0)
            n0 = b0 * 128
            bcnt = min(NB * 128, N - n0)
            xt = p1.tile([128, KI, NB * 128], BF16, tag="xt")
            for k in range(KI):
                nc.sync.dma_start(out=xt[:, k, :bcnt],
                                  in_=xT[k * 128:(k + 1) * 128, n0:n0 + bcnt])
            rsb = p1.tile([128, NB, ROWW], BF16, tag="rsb")
            nc.gpsimd.memset(rsb[:, :, MAINW:], 0)
            for j0 in range(0, bt, SB):
                sb = min(SB, bt - j0)
                rps = p1ps.tile([128, SB, AUXW], F32, tag="rps")
                for jj in range(sb):
                    j = j0 + jj
                    cnt = min(128, N - (b0 + j) * 128)
                    for k in range(KI):
                        nc.tensor.matmul(
                            out=rps[:cnt, jj, :],
                            lhsT=xt[:, k, j * 128:j * 128 + cnt],
                            rhs=aux_bf[:, k, :],
                            start=(k == 0), stop=(k == KI - 1))
                # main row payload: [el|g] = aux cols H..AUXW (Vector: idle
                # during phase 1, and the copy gates the rps pipeline)
                nc.vector.tensor_copy(out=rsb[:, j0:j0 + sb, :MAINW],
                                      in_=rps[:, :sb, H:AUXW])
                # own-range er extraction (rotated: own dst range = tiles 0..NW-1)
                t0 = b0 + j0
                if t0 < NW:
                    nt_er = min(sb, NW - t0)
                    nc.scalar.activation(
                        out=er_sb[:, t0 * H:(t0 + nt_er) * H]
                            .rearrange("p (w h) -> p w h", h=H),
                        in_=rps[:, :nt_er, 0:H], func=ACTF.Copy)
            if bcnt == bt * 128:
                out_ap = row_table[n0:n0 + bt * 128, :].rearrange(
                    "(j p) c -> p j c", p=128)
                nc.sync.dma_start(out=out_ap, in_=rsb[:, :bt, :])
            else:
                full = bcnt // 128
                if full:
                    out_ap = row_table[n0:n0 + full * 128, :].rearrange(
                        "(j p) c -> p j c", p=128)
                    nc.sync.dma_start(out=out_ap, in_=rsb[:, :full, :])
                for j in range(full, bt):
                    cnt = min(128, N - (b0 + j) * 128)
                    nc.sync.dma_start(
                        out=row_table[(b0 + j) * 128:(b0 + j) * 128 + cnt, :],
                        in_=rsb[:cnt, j, :])

        # ---------- phase 2: edge stream ----------
        cur_seg = [None]  # (group, half)
        gps_ref = [None]
        stage_ref = [None] * NG

        def stage_group(g):
            # h0 segment done: park partial sums in SBUF, free the PSUM bank
            stg = op.tile([128, GROUPW * MAINW], F32, tag="stage", bufs=NG,
                          name=f"stage{g}")
            stage_ref[g] = stg
            nc.scalar.activation(out=stg[:], in_=gps_ref[0][:], func=ACTF.Copy)

        def close_group(g):
            tot = op.tile([128, GROUPW * MAINW], F32, tag="tot")
            nc.vector.tensor_add(out=tot[:], in0=gps_ref[0][:],
                                 in1=stage_ref[g][:])
            gv = tot[:].rearrange("p (w c) -> p w c", c=MAINW)
            sg = op.tile([128, GROUPW * H], F32, tag="sg")
            nc.vector.tensor_scalar_max(out=sg[:], in0=gv[:, :, 0:H],
                                        scalar1=1e-30)
            rs = op.tile([128, GROUPW * H], F32, tag="rs")
            nc.vector.reciprocal(out=rs[:], in_=sg[:])
            ysb = op.tile([128, GROUPW * HD], F32, tag="ysb")
            nc.vector.tensor_tensor(
                out=ysb[:].rearrange("p (w h d) -> p w h d", h=H, d=D),
                in0=gv[:, :, H:MAINW].rearrange("p w (h d) -> p w h d", h=H),
                in1=rs[:].rearrange("p (w h) -> p w h", h=H)
                    .to_broadcast([128, GROUPW, H, D]),
                op=ALU.mult)
            nc.vector.tensor_add(out=ysb[:], in0=ysb[:], in1=brep_sb[:])
            n0 = g * GROUPW * WIN
            out_ap = y_t[n0:n0 + GROUPW * WIN, :].rearrange(
                "(w p) c -> p w c", p=128)
            nc.sync.dma_start(out=out_ap, in_=ysb[:].rearrange(
                "p (w c) -> p w c", c=HD))

        def end_segment(seg):
            if seg is None:
                return
            g, h = seg
            if h == 0:
                stage_group(g)
            else:
                close_group(g)

        nreg_cache = {}

        def nreg(n):
            if n not in nreg_cache:
                nreg_cache[n] = nc.gpsimd.to_reg(n)
            return nreg_cache[n]

        gq = [0]  # rotating SWDGE queue counter
        erq_ref = [None]  # 4-call-packed er PSUM bank

        for ci, (c0, ctiles, half) in enumerate(st.call_meta):
            ne = ctiles * 128
            abuf = gp.tile([128, CT, ROWW], BF16, tag="abuf")
            tab = row_table[half * cfg.SPLIT:(half + 1) * cfg.SPLIT, :]
            nc.gpsimd.dma_gather(abuf[:, :ctiles, :], tab,
                                 aidx_sb[:, c0 * 8:(c0 + ctiles) * 8],
                                 ctiles * 128, nreg(ctiles * 128), ROWW,
                                 queue_num=gq[0] % NQUEUES,
                                 single_packet=False)
            gq[0] += 1

            # batched one-hot S for this call (bf16)
            S_all = tp.tile([128, CT, WIN], BF16, tag="S", bufs=3)
            nc.vector.tensor_tensor(
                out=S_all[:, :ctiles, :],
                in0=dofc_sb[:, c0:c0 + ctiles]
                    .rearrange("p (t o) -> p t o", o=1)
                    .to_broadcast([128, ctiles, WIN]),
                in1=iota_f[:, :ctiles * WIN].rearrange("p (t w) -> p t w", w=WIN),
                op=ALU.is_equal)

            # quad-batched: ST = S^T via PE into one PSUM bank, one Scalar
            # cast per quad, then per-tile er fetch matmuls. er strips for
            # 4 calls share one PSUM bank (deeper pipeline lookahead).
            if ci % 4 == 0:
                erq_ref[0] = erps.tile([128, 4, CT * H], F32, tag="erps", name="erq")
            er_ps = erq_ref[0][:, ci % 4, :]
            for q0 in range(0, ctiles, 4):
                qn = min(4, ctiles - q0)
                st_ps = stpsp.tile([128, 4, 128], F32, tag="stps")
                for jj in range(qn):
                    nc.tensor.matmul(out=st_ps[:, jj, :],
                                     lhsT=S_all[:, q0 + jj, :],
                                     rhs=ident_sb[:], start=True, stop=True)
                st_sb = tp.tile([128, 4, 128], BF16, tag="stsb", bufs=8)
                nc.scalar.activation(out=st_sb[:, :qn, :], in_=st_ps[:, :qn, :],
                                     func=ACTF.Copy)
                for jj in range(qn):
                    j = q0 + jj
                    wv = st.tile_meta[c0 + j][0]
                    nc.tensor.matmul(out=er_ps[:, j * H:(j + 1) * H],
                                     lhsT=st_sb[:, jj, :],
                                     rhs=er_sb[:, wv * H:(wv + 1) * H],
                                     start=True, stop=True)

            # logits -> lrelu -> exp -> mgc (batched)
            esb = tp.tile([128, CT * H], BF16, tag="esb", bufs=3)
            nc.vector.tensor_tensor(
                out=esb[:, :ctiles * H].rearrange("p (t h) -> p t h", h=H),
                in0=abuf[:, :ctiles, 0:H],
                in1=er_ps[:, :ctiles * H].rearrange("p (t h) -> p t h", h=H),
                op=ALU.add)
            nc.vector.scalar_tensor_tensor(
                out=esb[:, :ctiles * H], in0=esb[:, :ctiles * H],
                scalar=NEG_SLOPE, in1=esb[:, :ctiles * H],
                op0=ALU.mult, op1=ALU.max)
            mgc = tp.tile([128, CT, MAINW], BF16, tag="mgc", bufs=3)
            nc.scalar.activation(out=mgc[:, :ctiles, 0:H],
                                 in_=esb[:, :ctiles * H]
                                 .rearrange("p (t h) -> p t h", h=H),
                                 func=ACTF.Exp)
            nc.vector.tensor_tensor(
                out=mgc[:, :ctiles, H:MAINW].rearrange("p t (h d) -> p t h d", h=H),
                in0=abuf[:, :ctiles, H:MAINW].rearrange("p t (h d) -> p t h d", h=H),
                in1=mgc[:, :ctiles, 0:H].to_broadcast([128, ctiles, H, D]),
                op=ALU.mult)

            # per-tile scatter matmuls into window-group accumulators
            for j in range(ctiles):
                tg = c0 + j
                wv, half_, first, last = st.tile_meta[tg]
                g = wv // GROUPW
                seg = (g, half_)
                if seg != cur_seg[0]:
                    end_segment(cur_seg[0])
                    gps_ref[0] = accp.tile([128, GROUPW * MAINW], F32, tag="gps",
                                           name="gps")
                    cur_seg[0] = seg
                gps = gps_ref[0]
                wloc = wv - g * GROUPW
                base = wloc * MAINW
                nc.tensor.matmul(out=gps[:, base:base + MAINW],
                                 lhsT=S_all[:, j, :], rhs=mgc[:, j, :],
                                 start=first, stop=last)
        end_segment(cur_seg[0])

    nc.compile()
    return nc


def make_in_maps(cfg, st, inputs, A_idx, doff):
    import ml_dtypes
    bf = ml_dtypes.bfloat16
    x = np.asarray(inputs["x"], np.float32)
    xT, WT, wcat, bias_flat, fcb_tiled = host_layouts(
        cfg, x, inputs["W"], inputs["attn_l"], inputs["attn_r"],
        inputs["bias"], inputs["fc_w"], inputs["fc_b"])

    def wrap16(a):  # [NPOS] -> [128, NPOS//16]
        return np.tile(np.ascontiguousarray(a.reshape(-1, 16).T), (8, 1))

    in_maps = []
    for c in range(cfg.NCORES):
        dof = doff[c]
        in_maps.append({
            "xT": np.ascontiguousarray(np.roll(xT, -c * cfg.NPC, axis=1)),
            "WT": WT, "wcat": wcat, "bias_flat": bias_flat,
            "fcb_tiled": fcb_tiled,
            "A_idx": wrap16(A_idx[c]),
            "dofc": np.ascontiguousarray(dof.reshape(-1, 128).T).astype(bf),
            "iota_rep": np.tile(np.arange(cfg.WIN), (128, cfg.CT)).astype(bf),
            "ident": np.eye(128, dtype=np.float32).astype(bf),
        })
    return in_maps


def kernel(**inputs):
    import numpy as np
    from concourse import bass_utils

    cfg = Cfg()
    src = np.asarray(inputs["src"])
    dst = np.asarray(inputs["dst"])
    assert src.shape == (cfg.E,) and dst.shape == (cfg.E,)
    st, A_idx, doff = preprocess(cfg, src, dst)
    nc = build_program(cfg, st)
    in_maps = make_in_maps(cfg, st, inputs, A_idx, doff)
    res = bass_utils.run_bass_kernel_spmd(
        nc, in_maps, core_ids=list(range(cfg.NCORES)))
    y = np.concatenate([r["y"][:cfg.NPC] for r in res.results], axis=0)
    return np.ascontiguousarray(y.reshape(cfg.N, cfg.H, cfg.D).astype(np.float32))
